# revision 45
# baseline (speedup 1.0000x reference)
"""Trainium2 Bass kernel for a 2-layer GAT occupancy predictor (B=1).

Reference math:
  pts = concat(pos, pos_non_manifold) -> [K=6000, 3]
  mask[i,j] = ||pts_i - pts_j||^2 < 0.05^2          (dense radius graph)
  layer l:  h = x @ Wl                              [K, 4*64]
            e[i,j,h] = leaky02(ed[i,h] + es[j,h])   es/ed = <h, a_src/dst>
            alpha = softmax_j(e masked)
            x' = relu(alpha @ h + b)
  logits = (x2 @ fc_w + fc_b)[M:] reshaped to [1, 2, 3000]

Distribution (8 NeuronCores): nodes are Morton-sorted; core c owns the 768
destinations [768c, 768(c+1)) of the padded 6144-node graph.  Each core's
sources are CUSTOM-PACKED: only the ~900 nodes within radius of its block,
gathered into T=ceil(max_unique/128) tiles of 128 (padded with node 6143),
instead of whole global Morton tiles.  This cuts per-core source tiles from
~28 to ~8 and makes dense-768-dst processing cheap enough to skip chunking.

Everything 16-bit on the hot path (fp16), f32 accumulation in PSUM:
  per slot s (128 sources x 768 dsts x 4 heads):
    PE   : layer1 h = p @ W1 [128,256]; g = (R2-d2 | es-cols) via one K=5
           matmul; transposed aggregation x^T[c,dst] += A.h with [h|ones]
           stationary (denominator rides as the 65th weight column) in
           1KB-aligned 256-col chunks, one start=True per PSUM bank
           (start clears has_written for the WHOLE bank).
    DVE  : mask thresholds mn = (g<0)*-60000 (psum->fp16); ONE 2x TT
           u4 = ed + mn for all heads via a stride-0 broadcast AP of mn;
           heads 2-3 es-adds (4x TS) + leaky as TS-mul + TT-max.
    ACT  : heads 0-1 leaky via Prelu(u4, bias=es); one exp over
           [128, 4*768]; 1/den as exp(-ln(den)) straight off PSUM.
    gpsimd: ed/deninv partition broadcasts, layer-2 remote-row gathers.
  Between layers: x1^T assembled by 4 partition-moving DMAs; h2 = x1 @ W2
  (+es ride-along) computed per-owner, AllGathered as fp16 node-major rows
  [h0|1|h1|1|h2|1|h3|1|es4].  Each core's first 768 sources are its own
  nodes in identity order, so layer-2 slots 0-5 read h2 rows straight from
  the resident hg_sb buffer and overlap the whole AllGather; only the 1-2
  remote slots wait for it.  Masks bounce through DRAM between layers.
"""

import sys

sys.path.insert(0, "/opt/trn_rl_repo")

from contextlib import ExitStack

import ml_dtypes
import numpy as np

import concourse.bacc as bacc
import concourse.bass as bass
import concourse.mybir as mybir
import concourse.tile as tile
from concourse.bass_utils import run_bass_kernel_spmd

F32 = mybir.dt.float32
F16 = mybir.dt.float16
I32 = mybir.dt.int32
AF = mybir.ActivationFunctionType
OP = mybir.AluOpType
AX = mybir.AxisListType

N_CORES = 8
N = 3000
M = 3000
K = N + M          # real nodes
KP = 6144          # padded nodes
IC = KP // N_CORES # 768 destinations per core
H = 4              # heads
C = 64             # channels per head
HC = H * C         # 256
HCE = HC + H       # 260: h columns + es columns (layer-2 ride-along)
ROWW = H * (C + 1) + H  # 264: AG row [h0|1|h1|1|h2|1|h3|1|es4]
R2 = float(np.float32(0.05) * np.float32(0.05))
PAD_COORD = -1.0
PAD_NODE = KP - 1
MASK_EPS = 1e-5    # host activity-test margin (superset of device mask)
MNEG = -60000.0    # masked-score offset; *0.2 then exp -> 0 in fp16
GA = 384           # d2/mask column chunk (PSUM bank budget)


def build(nslot, nexp, n_cores=N_CORES, fake_ag=False, dbg=False):
    nc = bacc.Bacc("TRN2", target_bir_lowering=False, debug=False,
                   num_devices=n_cores)
    T = nslot
    E = nexp
    dbg_d = {}
    if dbg:
        for nm, shp, dt in (("dbg_den", [1, H * IC], F32),
                            ("dbg_dinv", [1, H * IC], F32),
                            ("dbg_x1T", [128, 2, IC], F16),
                            ("dbg_edb", [128, H, IC], F16),
                            ("dbg_mn0", [128, IC], F16),
                            ("dbg_A0", [128, H, IC], F16),
                            ("dbg_hsrc", [128, nslot, ROWW], F16)):
            dbg_d[nm] = nc.dram_tensor(nm, shp, dt, kind="ExternalOutput")

    # ---- kernel I/O (identical program on every core) ----
    sel5_d = nc.dram_tensor("sel5", [5, T * 128], F32, kind="ExternalInput")
    # own5ge: cols 0:768 = [2p; -1; R2-sq] (g = R2-d2), cols 768:772 = es1
    own5ge_d = nc.dram_tensor("own5ge", [5, IC + H], F32,
                              kind="ExternalInput")
    own3_d = nc.dram_tensor("own3", [3, IC], F32, kind="ExternalInput")
    agidx_d = nc.dram_tensor("agidx", [128, T], I32, kind="ExternalInput")
    expdst_d = nc.dram_tensor("expdst", [128, IC // 128], I32,
                              kind="ExternalInput")
    w1p_d = nc.dram_tensor("w1p", [3, HC], F32, kind="ExternalInput")
    w1d_d = nc.dram_tensor("w1d", [3, H], F32, kind="ExternalInput")
    w2p_d = nc.dram_tensor("w2p", [HC, HCE], F16, kind="ExternalInput")
    admw2_d = nc.dram_tensor("admw2", [C, H, H], F16, kind="ExternalInput")
    b1t_d = nc.dram_tensor("b1t", [C, H], F32, kind="ExternalInput")
    b2t_d = nc.dram_tensor("b2t", [C, H], F32, kind="ExternalInput")
    fcw_d = nc.dram_tensor("fcw", [C, H, 2], F16, kind="ExternalInput")
    fcb_d = nc.dram_tensor("fcb", [128, 2], F32, kind="ExternalInput")

    out_d = nc.dram_tensor("out", [IC, 2], F32, kind="ExternalOutput")

    with tile.TileContext(nc) as tc, ExitStack() as st:
        dram = st.enter_context(tc.tile_pool(name="dram", bufs=1,
                                             space="DRAM"))
        mn_dram = dram.tile([T, 128, IC], F16)
        edt_dram = dram.tile([H * IC], F16)
        hg_dram = dram.tile([IC, ROWW], F16)
        ag_out = dram.tile([KP, ROWW], F16,
                           addr_space=("Local" if fake_ag else "Shared"))

        const = st.enter_context(tc.tile_pool(name="const", bufs=1))
        sel5_sb = const.tile([5, T * 128], F32)
        own5ge_sb = const.tile([5, IC + H], F32)
        own3_sb = const.tile([3, IC], F32)
        agidx_sb = const.tile([128, T], I32)
        expdst_sb = const.tile([128, IC // 128], I32)
        w1p_sb = const.tile([3, HC], F32)
        w1d_sb = const.tile([3, H], F32)
        w2p_sb = const.tile([128, 2, HCE], F16)
        admw2_sb = const.tile([C, H, H], F16)
        b1t_sb = const.tile([C, H], F32)
        b2t_sb = const.tile([C, H], F32)
        fcw_sb = const.tile([C, H, 2], F16)
        fcb_sb = const.tile([128, 2], F32)

        nc.sync.dma_start(out=own3_sb[:, :], in_=own3_d[:, :])
        nc.sync.dma_start(out=w1d_sb[:, :], in_=w1d_d[:, :])
        nc.sync.dma_start(out=sel5_sb[:, :], in_=sel5_d[:, :])
        nc.sync.dma_start(out=own5ge_sb[:, :], in_=own5ge_d[:, :])
        nc.sync.dma_start(out=w1p_sb[:, :], in_=w1p_d[:, :])
        nc.sync.dma_start(out=agidx_sb[:, :], in_=agidx_d[:, :])
        nc.sync.dma_start(out=expdst_sb[:, :], in_=expdst_d[:, :])
        nc.sync.dma_start(out=w2p_sb[:, :, :],
                          in_=w2p_d.rearrange("(s p) c -> p s c", p=128))
        nc.sync.dma_start(out=admw2_sb[:, :, :], in_=admw2_d[:, :, :])
        nc.sync.dma_start(out=b1t_sb[:, :], in_=b1t_d[:, :])
        nc.sync.dma_start(out=b2t_sb[:, :], in_=b2t_d[:, :])
        nc.sync.dma_start(out=fcw_sb[:, :, :], in_=fcw_d[:, :, :])
        nc.sync.dma_start(out=fcb_sb[:, :], in_=fcb_d[:, :])

        big = st.enter_context(tc.tile_pool(name="big", bufs=1))
        # layer-1 source features, AG-row layout [h0|1|h1|1|h2|1|h3|1|es4]
        hsrc = big.tile([128, T, ROWW], F16)
        es4f = big.tile([128, T, H], F32)
        ed_b = big.tile([128, H, IC], F16)
        edt_sb = big.tile([H, IC], F16)
        edt_row = big.tile([1, H, IC], F16)
        x1T = big.tile([128, 2, IC], F16)
        x2T = big.tile([128, 2, IC], F16)
        hg_sb = big.tile([128, IC // 128, ROWW], F16)
        xr = big.tile([C, H, IC], F16)
        den_sb = big.tile([128, H * IC], F32)
        dinv_sb = big.tile([128, H * IC], F32)
        dinv_row = big.tile([1, H * IC], F32)
        dinv_b = big.tile([128, H, IC], F32)
        logit_sb = big.tile([128, IC // 128, 2], F32)

        h65 = hsrc[:, :, 0:H * (C + 1)].rearrange("p t (h x) -> p t h x", h=H)
        nc.vector.memset(h65[:, :, :, C:C + 1], 1.0)
        g65 = hg_sb[:, :, 0:H * (C + 1)].rearrange("p q (h x) -> p q h x",
                                                   h=H)
        nc.vector.memset(g65[:, :, :, C:C + 1], 1.0)

        for layer in (1, 2):
            # ---- prep: edt rows + partition-broadcast to ed_b ----
            with tc.tile_pool(name=f"prep{layer}", bufs=1,
                              space="PSUM") as prep_ps:
                edt_ps = prep_ps.tile([H, IC], F32, tag="edt")
                for lo, sz in ((0, 512), (512, 256)):
                    sl = slice(lo, lo + sz)
                    if layer == 1:
                        nc.tensor.matmul(edt_ps[:, sl], w1d_sb[:, :],
                                         own3_sb[:, sl],
                                         start=True, stop=True)
                    else:
                        for hh in range(H):
                            nc.tensor.matmul(edt_ps[:, sl],
                                             admw2_sb[:, hh, :],
                                             xr[0:C, hh, sl],
                                             start=(hh == 0),
                                             stop=(hh == H - 1))
                nc.scalar.copy(edt_sb[:, :], edt_ps[:, :])
            for h in range(H):
                nc.sync.dma_start(out=edt_row[0:1, h, :],
                                  in_=edt_sb[h:h + 1, :])
            for h in range(H):
                nc.gpsimd.partition_broadcast(ed_b[:, h, :],
                                              edt_row[0:1, h, :])

            # ---- slot loop ----
            with tc.tile_pool(name=f"agg_ps{layer}", bufs=1,
                              space="PSUM") as agg_pool:
                agg_ps = agg_pool.tile([128, H, IC], F32, tag="agg",
                                       name=f"agg_{layer}")
                with tc.tile_pool(name=f"jl{layer}", bufs=4) as jl, \
                     tc.tile_pool(name=f"mnp{layer}", bufs=3) as mnp, \
                     tc.tile_pool(name=f"h_ps{layer}", bufs=1,
                                  space="PSUM") as h_psp:
                    for s in range(T):
                        mn = mnp.tile([128, IC], F16, tag="mn",
                                      name=f"mn_{layer}_{s}")
                        if layer == 1:
                            h_ps = h_psp.tile([128, HC], F32, tag="h",
                                              name=f"h_ps_{s}")
                            nc.tensor.matmul(
                                h_ps[:, :],
                                sel5_sb[0:3, s * 128:(s + 1) * 128],
                                w1p_sb[:, :], start=True, stop=True)
                            nc.scalar.copy(
                                h65[:, s, :, 0:C],
                                h_ps[:, :].rearrange("p (h c) -> p h c",
                                                     h=H))
                            # g = R2-d2 (+es cols on 2nd chunk), 2 chunks
                            g_ps = h_psp.tile([128, GA + H], F32, tag="g",
                                              name=f"g_ps_{s}")
                            nc.tensor.matmul(
                                g_ps[:, 0:GA],
                                sel5_sb[:, s * 128:(s + 1) * 128],
                                own5ge_sb[:, 0:GA], start=True, stop=True)
                            nc.vector.tensor_scalar(
                                mn[:, 0:GA], g_ps[:, 0:GA], 0.0, MNEG,
                                OP.is_lt, OP.mult)
                            nc.tensor.matmul(
                                g_ps[:, :],
                                sel5_sb[:, s * 128:(s + 1) * 128],
                                own5ge_sb[:, GA:IC + H],
                                start=True, stop=True)
                            nc.vector.tensor_scalar(
                                mn[:, GA:IC], g_ps[:, 0:IC - GA], 0.0, MNEG,
                                OP.is_lt, OP.mult)
                            nc.vector.tensor_scalar_add(
                                es4f[:, s, :],
                                g_ps[:, IC - GA:IC - GA + H], 0.0)
                            nc.sync.dma_start(out=mn_dram[s, :, :],
                                              in_=mn[:, :])
                            src = hsrc[:, s, :]
                            es_ap = es4f[:, s, :]
                        else:
                            if s < IC // 128:
                                # own-node slots: sources are exactly this
                                # core's nodes s*128..s*128+127, whose h2
                                # rows already sit in hg_sb -- no gather,
                                # no dependency on the AllGather.
                                src = hg_sb[:, s, :]
                            else:
                                src = jl.tile([128, ROWW], F16, tag="hg",
                                              name=f"hg_{s}")
                                nc.gpsimd.indirect_dma_start(
                                    out=src[:, :], out_offset=None,
                                    in_=ag_out[:, :],
                                    in_offset=bass.IndirectOffsetOnAxis(
                                        ap=agidx_sb[:, s:s + 1], axis=0))
                            nc.sync.dma_start(out=mn[:, :],
                                              in_=mn_dram[s, :, :])
                            esg = jl.tile([128, H], F32, tag="esg",
                                          name=f"esg_{s}")
                            nc.vector.tensor_scalar_add(
                                esg[:, :],
                                src[:, H * (C + 1):ROWW], 0.0)
                            es_ap = esg[:, :]

                        # scores: L = leaky02(ed + es + mn); A = exp(L).
                        # u4 = ed + mn in ONE 2x TT via a stride-0 head
                        # broadcast of mn.  Then heads 0-1 get es+leaky via
                        # ACT Prelu (bias=es); heads 2-3 via 4x TS es-adds
                        # and a TS/TT leaky (STT only has a 1x uop).
                        L4 = jl.tile([128, H, IC], F16, tag="L4",
                                     name=f"L4_{layer}_{s}")
                        u4 = jl.tile([128, H, IC], F16, tag="u4",
                                     name=f"u4_{layer}_{s}")
                        ub, mb = bass.broadcast_tensor_aps(
                            ed_b[:, :, :],
                            mn[:, :].rearrange("p (o d) -> p o d", o=1))
                        nc.vector.tensor_tensor(u4[:, :, :], ub, mb, OP.add)
                        for h in range(2):
                            nc.scalar.activation(
                                L4[:, h, :], u4[:, h, :], AF.Prelu,
                                bias=es_ap[:, h:h + 1],
                                scale=1.0, alpha=0.2)
                        v2 = jl.tile([128, 2, IC], F16, tag="v2",
                                     name=f"v2_{layer}_{s}")
                        for h in range(2, H):
                            nc.vector.tensor_scalar_add(
                                v2[:, h - 2, :], u4[:, h, :],
                                es_ap[:, h:h + 1])
                        t2 = jl.tile([128, 2, IC], F16, tag="t2",
                                     name=f"t2_{layer}_{s}")
                        nc.vector.tensor_scalar_mul(t2[:, :, :], v2[:, :, :],
                                                    0.2)
                        nc.vector.tensor_tensor(L4[:, 2:4, :], v2[:, :, :],
                                                t2[:, :, :], OP.max)
                        A4 = jl.tile([128, H, IC], F16, tag="A4",
                                     name=f"A4_{layer}_{s}")
                        nc.scalar.activation(A4[:, :, :], L4[:, :, :], AF.Exp)
                        if dbg and layer == 1 and s == 0:
                            nc.sync.dma_start(out=dbg_d["dbg_mn0"][:, :],
                                              in_=mn[:, :])
                            nc.sync.dma_start(out=dbg_d["dbg_A0"][:, :, :],
                                              in_=A4[:, :, :])

                        # transposed aggregation: [h|ones] stationary.
                        # 256-col (1KB) chunks keep every matmul output
                        # inside one PSUM bank (head stride is 3KB).
                        # start=True clears has_written for the WHOLE bank,
                        # so only the first-issued region of each bank may
                        # set it (those with q ≡ h mod 2); the bank-mate
                        # region's first write then lands in overwrite mode.
                        for h in range(H):
                            for q in range(3):
                                lo = q * 256
                                nc.tensor.matmul(
                                    agg_ps[0:C + 1, h, lo:lo + 256],
                                    src[:, h * (C + 1):(h + 1) * (C + 1)],
                                    A4[:, h, lo:lo + 256],
                                    start=(s == 0 and (q % 2) == (h % 2)),
                                    stop=(s == T - 1))

                # ---- finalize: x^T = relu(num*dinv + b) ----
                bt_sb = b1t_sb if layer == 1 else b2t_sb
                xT = x1T if layer == 1 else x2T
                with tc.tile_pool(name=f"fin{layer}", bufs=1) as fin:
                    # den: PSUM -> SBUF row -> [128,:] for a lane-parallel
                    # exact reciprocal -> partition-0 row -> broadcast.
                    # Processed per head-pair so the two halves pipeline.
                    xc = fin.tile([C, H, IC], F16, tag="xc")
                    HF = 2 * IC  # elements per head-pair
                    for g2 in range(2):
                        hs = slice(2 * g2, 2 * g2 + 2)
                        fs = slice(g2 * HF, (g2 + 1) * HF)
                        # 1/den as exp(-ln(den)): two ACT table ops straight
                        # from PSUM beat the DVE iterative divide ~5x here
                        nc.scalar.activation(
                            den_sb[C:C + 1, fs],
                            agg_ps[C:C + 1, hs, :].rearrange(
                                "p h d -> p (h d)"),
                            AF.Ln)
                        nc.scalar.activation(
                            dinv_sb[C:C + 1, fs], den_sb[C:C + 1, fs],
                            AF.Exp, scale=-1.0)
                        nc.sync.dma_start(out=dinv_row[0:1, fs],
                                          in_=dinv_sb[C:C + 1, fs])
                        nc.gpsimd.partition_broadcast(
                            dinv_b[0:C, hs, :].rearrange(
                                "p h d -> p (h d)"),
                            dinv_row[0:1, fs])
                        nc.vector.tensor_tensor(
                            xc[:, hs, :], agg_ps[0:C, hs, :],
                            dinv_b[0:C, hs, :], OP.mult)
                        for h in range(2 * g2, 2 * g2 + 2):
                            nc.vector.tensor_scalar(
                                xr[:, h, :], xc[:, h, :], bt_sb[:, h:h + 1],
                                0.0, OP.add, OP.max)
                            po = (h % 2) * C
                            nc.sync.dma_start(
                                out=xT[po:po + C, h // 2, :],
                                in_=xr[0:C, h, :])

            if dbg and layer == 1:
                nc.sync.dma_start(out=dbg_d["dbg_x1T"][:, :, :],
                                  in_=x1T[:, :, :])
                nc.sync.dma_start(out=dbg_d["dbg_edb"][:, :, :],
                                  in_=ed_b[:, :, :])
                nc.sync.dma_start(out=dbg_d["dbg_hsrc"][:, :, :],
                                  in_=hsrc[:, :, :])
            if layer == 1:
                # ---- h2 rows (+es) for all own nodes; AllGather ----
                with tc.tile_pool(name="h2", bufs=1, space="PSUM") as h2p:
                    # half 0 (heads 0-1 of x1T) is ready after the first
                    # finalize half -- issue all 6 chunks' first matmuls
                    # immediately, accumulate half 1 when it lands
                    h2_tiles = [h2p.tile([128, HCE], F32, tag=f"h2_{oc}",
                                         name=f"h2_{oc}")
                                for oc in range(IC // 128)]
                    for s2 in range(2):
                        for oc in range(IC // 128):
                            nc.tensor.matmul(
                                h2_tiles[oc][:, :],
                                x1T[:, s2, oc * 128:(oc + 1) * 128],
                                w2p_sb[:, s2, :],
                                start=(s2 == 0), stop=(s2 == 1))
                    for oc in range(IC // 128):
                        nc.scalar.copy(
                            g65[:, oc, :, 0:C],
                            h2_tiles[oc][:, 0:HC].rearrange(
                                "p (h c) -> p h c", h=H))
                        nc.vector.tensor_scalar_add(
                            hg_sb[:, oc, H * (C + 1):ROWW],
                            h2_tiles[oc][:, HC:HCE], 0.0)
                nc.sync.dma_start(
                    out=hg_dram.rearrange("(q p) r -> p q r", p=128),
                    in_=hg_sb[:, :, :])
                if fake_ag:
                    for r in range(n_cores):
                        nc.sync.dma_start(
                            out=ag_out[r * IC:(r + 1) * IC, :],
                            in_=hg_dram[:, :])
                else:
                    nc.gpsimd.collective_compute(
                        "AllGather", OP.bypass,
                        replica_groups=[list(range(n_cores))],
                        ins=[hg_dram.opt()],
                        outs=[ag_out.opt()])
            else:
                # ---- fc head: per (head, chunk) straight from xr ----
                with tc.tile_pool(name="fc", bufs=1, space="PSUM") as fcp:
                    logit_ps = fcp.tile([128, IC // 128, 2], F32, tag="lg")
                    # all chunks share one PSUM bank: single start=True
                    for h in range(H):
                        for oc in range(IC // 128):
                            nc.tensor.matmul(
                                logit_ps[:, oc, :],
                                xr[0:C, h, oc * 128:(oc + 1) * 128],
                                fcw_sb[:, h, :],
                                start=(h == 0 and oc == 0), stop=(h == H - 1))
                    for o in range(2):
                        nc.vector.tensor_scalar_add(
                            logit_sb[:, :, o], logit_ps[:, :, o],
                            fcb_sb[:, o:o + 1])
                nc.sync.dma_start(
                    out=out_d.rearrange("(q p) o -> p q o", p=128),
                    in_=logit_sb[:, :, :])

    nc.compile()
    return nc


_BUILD_CACHE = {}


def _get_nc(nslot, nexp):
    key = (nslot, nexp)
    if key not in _BUILD_CACHE:
        _BUILD_CACHE[key] = build(nslot, nexp)
    return _BUILD_CACHE[key]


def _morton(p, bits=10):
    q = np.clip((p * (1 << bits)).astype(np.int64), 0, (1 << bits) - 1)
    code = np.zeros(len(p), np.int64)
    for b in range(bits):
        for dim in range(3):
            code |= ((q[:, dim] >> b) & 1) << (3 * b + dim)
    return code


def _plan(pts):
    """Sort nodes spatially; pick each core's relevant-source node list."""
    order = np.argsort(_morton(pts), kind="stable")
    p_sorted = np.full((KP, 3), PAD_COORD, np.float32)
    p_sorted[:K] = pts[order]

    sq = (p_sorted ** 2).sum(-1, dtype=np.float32)
    G = p_sorted @ p_sorted.T
    d2 = sq[None, :] + sq[:, None] - 2.0 * G
    near = d2 < (R2 + MASK_EPS)          # [src, dst], conservative superset

    srcs_list = []
    for c in range(N_CORES):
        srcs = np.flatnonzero(near[:, c * IC:(c + 1) * IC].any(axis=1))
        # own nodes first in identity order (they are always all present
        # via self-edges); remote sources after.  Slots 0..5 then read
        # their rows straight out of the resident hg_sb buffer.
        own = np.arange(c * IC, (c + 1) * IC, dtype=srcs.dtype)
        rem = srcs[(srcs < c * IC) | (srcs >= (c + 1) * IC)]
        srcs_list.append(np.concatenate([own, rem]))
    T = max(-(-len(s) // 128) for s in srcs_list)
    srcs_list = [np.concatenate(
        [s, np.full(T * 128 - len(s), PAD_NODE, s.dtype)])
        for s in srcs_list]
    # export sets: rows of owner o consumed by any other core
    exp_sets = [set() for _ in range(N_CORES)]
    for c in range(N_CORES):
        s = srcs_list[c]
        rem = s[(s != PAD_NODE) & ((s < c * IC) | (s >= (c + 1) * IC))]
        for r in rem:
            exp_sets[int(r) // IC].add(int(r))
    exp_rows = [np.array(sorted(e), np.int64) for e in exp_sets]
    E = max(8, max(len(e) for e in exp_rows))
    return order, p_sorted, srcs_list, T, exp_rows, E


def _blockdiag(a):  # [H, C] -> [HC, H] fp32
    m = np.zeros((HC, H), dtype=np.float32)
    for h in range(H):
        m[h * C:(h + 1) * C, h] = np.asarray(a, np.float32)[h]
    return m


def _prep_inputs(pos, pos_non_manifold, W1, a_src1, a_dst1, b1,
                 W2, a_src2, a_dst2, b2, fc_w, fc_b):
    f16 = np.float16
    pts = np.concatenate([np.asarray(pos, np.float32),
                          np.asarray(pos_non_manifold, np.float32)],
                         axis=2)[0].T  # [K, 3]
    order, p_sorted, srcs_list, T, exp_rows, E = _plan(pts)
    sq_sorted = (p_sorted ** 2).sum(-1, dtype=np.float32)
    # global node id -> AllGather row position (owner-block concat of exports)
    ag_pos = np.full(KP, 0, np.int64)
    for o in range(N_CORES):
        ag_pos[exp_rows[o]] = o * E + np.arange(len(exp_rows[o]))

    W1f = np.asarray(W1, np.float32)
    W2f = np.asarray(W2, np.float32)
    w1s = W1f @ _blockdiag(a_src1)            # [3, H]
    w2p = np.concatenate([W2f, W2f @ _blockdiag(a_src2)], axis=1)

    shared = {
        "w1p": np.ascontiguousarray(W1f),
        "w1d": np.ascontiguousarray(W1f @ _blockdiag(a_dst1)),
        "w2p": np.ascontiguousarray(w2p.astype(f16)),
        "admw2": np.ascontiguousarray(
            (W2f @ _blockdiag(a_dst2)).reshape(H, C, H).transpose(
                1, 0, 2).astype(f16)),
        "b1t": np.ascontiguousarray(
            np.asarray(b1, np.float32).reshape(H, C).T),
        "b2t": np.ascontiguousarray(
            np.asarray(b2, np.float32).reshape(H, C).T),
        "fcw": np.ascontiguousarray(np.asarray(fc_w, np.float32).reshape(
            H, C, 2).transpose(1, 0, 2).astype(f16)),
        "fcb": np.ascontiguousarray(np.broadcast_to(
            np.asarray(fc_b, np.float32).reshape(1, 2), (128, 2))),
    }
    in_maps = []
    for c in range(N_CORES):
        srcs = srcs_list[c]
        psel = p_sorted[srcs]                     # [T*128, 3]
        pown = p_sorted[c * IC:(c + 1) * IC]
        sel5 = np.concatenate(
            [psel.T, sq_sorted[srcs][None, :],
             np.ones((1, len(srcs)), np.float32)], axis=0)
        own5 = np.concatenate(
            [2.0 * pown.T, -np.ones((1, IC), np.float32),
             (R2 - sq_sorted[c * IC:(c + 1) * IC])[None, :]], axis=0)
        es_cols = np.concatenate(
            [w1s, np.zeros((2, H), np.float32)], axis=0)  # [5, H]
        m = dict(shared)
        m["sel5"] = np.ascontiguousarray(sel5)
        m["own5ge"] = np.ascontiguousarray(
            np.concatenate([own5, es_cols], axis=1))
        m["own3"] = np.ascontiguousarray(pown.T)
        m["agidx"] = np.ascontiguousarray(
            srcs.reshape(T, 128).T.astype(np.int32))
        m["expdst"] = np.zeros((128, IC // 128), np.int32)
        in_maps.append(m)
    return in_maps, order, T, E


def kernel(pos, pos_non_manifold, W1, a_src1, a_dst1, b1,
           W2, a_src2, a_dst2, b2, fc_w, fc_b, _trace=False):
    in_maps, order, T, E = _prep_inputs(
        pos, pos_non_manifold, W1, a_src1, a_dst1, b1,
        W2, a_src2, a_dst2, b2, fc_w, fc_b)
    nc = _get_nc(T, E)
    res = run_bass_kernel_spmd(nc, in_maps, core_ids=list(range(N_CORES)),
                               trace=_trace)
    kernel.last_results = res
    x2s = np.concatenate([res.results[c]["out"] for c in range(N_CORES)],
                         axis=0)  # [KP, 2] in sorted order
    x2 = np.empty((K, 2), np.float32)
    x2[order] = x2s[:K]
    logits = np.ascontiguousarray(x2[M:K]).reshape(1, 2, 3000)
    return logits.astype(np.float32)


# revision 46
# speedup vs baseline: 1.1039x; 1.1039x over previous
"""Trainium2 Bass kernel for a 2-layer GAT occupancy predictor (B=1).

Reference math:
  pts = concat(pos, pos_non_manifold) -> [K=6000, 3]
  mask[i,j] = ||pts_i - pts_j||^2 < 0.05^2          (dense radius graph)
  layer l:  h = x @ Wl                              [K, 4*64]
            e[i,j,h] = leaky02(ed[i,h] + es[j,h])   es/ed = <h, a_src/dst>
            alpha = softmax_j(e masked)
            x' = relu(alpha @ h + b)
  logits = (x2 @ fc_w + fc_b)[M:] reshaped to [1, 2, 3000]

Distribution (8 NeuronCores): nodes are Morton-sorted; core c owns the 768
destinations [768c, 768(c+1)) of the padded 6144-node graph.  Each core's
sources are CUSTOM-PACKED: only the ~900 nodes within radius of its block,
gathered into T=ceil(max_unique/128) tiles of 128 (padded with node 6143),
instead of whole global Morton tiles.  This cuts per-core source tiles from
~28 to ~8 and makes dense-768-dst processing cheap enough to skip chunking.

Everything 16-bit on the hot path (fp16), f32 accumulation in PSUM:
  per slot s (128 sources x 768 dsts x 4 heads):
    PE   : layer1 h = p @ W1 [128,256]; g = (R2-d2 | es-cols) via one K=5
           matmul; transposed aggregation x^T[c,dst] += A.h with [h|ones]
           stationary (denominator rides as the 65th weight column) in
           1KB-aligned 256-col chunks, one start=True per PSUM bank
           (start clears has_written for the WHOLE bank).
    DVE  : mask thresholds mn = (g<0)*-60000 (psum->fp16); ONE 2x TT
           u4 = ed + mn for all heads via a stride-0 broadcast AP of mn;
           heads 2-3 es-adds (4x TS) + leaky as TS-mul + TT-max.
    ACT  : heads 0-1 leaky via Prelu(u4, bias=es); one exp over
           [128, 4*768]; 1/den as exp(-ln(den)) straight off PSUM.
    gpsimd: ed/deninv partition broadcasts, layer-2 remote-row gathers.
  Between layers: x1^T assembled by 4 partition-moving DMAs; h2 = x1 @ W2
  (+es ride-along) computed per-owner, AllGathered as fp16 node-major rows
  [h0|1|h1|1|h2|1|h3|1|es4].  Each core's first 768 sources are its own
  nodes in identity order, so layer-2 slots 0-5 read h2 rows straight from
  the resident hg_sb buffer and overlap the whole AllGather; only the 1-2
  remote slots wait for it.  Masks bounce through DRAM between layers.
"""

import sys

sys.path.insert(0, "/opt/trn_rl_repo")

from contextlib import ExitStack

import ml_dtypes
import numpy as np

import concourse.bacc as bacc
import concourse.bass as bass
import concourse.mybir as mybir
import concourse.tile as tile
from concourse.bass_utils import run_bass_kernel_spmd

F32 = mybir.dt.float32
F16 = mybir.dt.float16
I32 = mybir.dt.int32
AF = mybir.ActivationFunctionType
OP = mybir.AluOpType
AX = mybir.AxisListType

N_CORES = 8
N = 3000
M = 3000
K = N + M          # real nodes
KP = 6144          # padded nodes
IC = KP // N_CORES # 768 destinations per core
H = 4              # heads
C = 64             # channels per head
HC = H * C         # 256
HCE = HC + H       # 260: h columns + es columns (layer-2 ride-along)
ROWW = H * (C + 1) + H  # 264: AG row [h0|1|h1|1|h2|1|h3|1|es4]
R2 = float(np.float32(0.05) * np.float32(0.05))
PAD_COORD = -1.0
PAD_NODE = KP - 1
MASK_EPS = 1e-5    # host activity-test margin (superset of device mask)
MNEG = -60000.0    # masked-score offset; *0.2 then exp -> 0 in fp16
GA = 384           # d2/mask column chunk (PSUM bank budget)


def build(nslot, nexp, n_cores=N_CORES, fake_ag=False, dbg=False):
    nc = bacc.Bacc("TRN2", target_bir_lowering=False, debug=False,
                   num_devices=n_cores)
    T = nslot
    E = nexp
    dbg_d = {}
    if dbg:
        for nm, shp, dt in (("dbg_den", [1, H * IC], F32),
                            ("dbg_dinv", [1, H * IC], F32),
                            ("dbg_x1T", [128, 2, IC], F16),
                            ("dbg_edb", [128, H, IC], F16),
                            ("dbg_mn0", [128, IC], F16),
                            ("dbg_A0", [128, H, IC], F16),
                            ("dbg_hsrc", [128, nslot, ROWW], F16)):
            dbg_d[nm] = nc.dram_tensor(nm, shp, dt, kind="ExternalOutput")

    # ---- kernel I/O (identical program on every core) ----
    sel5_d = nc.dram_tensor("sel5", [5, T * 128], F32, kind="ExternalInput")
    # own5ge: cols 0:768 = [2p; -1; R2-sq] (g = R2-d2), cols 768:772 = es1
    own5ge_d = nc.dram_tensor("own5ge", [5, IC + H], F32,
                              kind="ExternalInput")
    own3_d = nc.dram_tensor("own3", [3, IC], F32, kind="ExternalInput")
    agidx_d = nc.dram_tensor("agidx", [128, T], I32, kind="ExternalInput")
    expdst_d = nc.dram_tensor("expdst", [128, IC // 128], I32,
                              kind="ExternalInput")
    w1p_d = nc.dram_tensor("w1p", [3, HC], F32, kind="ExternalInput")
    w1d_d = nc.dram_tensor("w1d", [3, H], F32, kind="ExternalInput")
    w2p_d = nc.dram_tensor("w2p", [HC, HCE], F16, kind="ExternalInput")
    admw2_d = nc.dram_tensor("admw2", [HC, H], F16, kind="ExternalInput")
    b1t_d = nc.dram_tensor("b1t", [C, H], F32, kind="ExternalInput")
    b2t_d = nc.dram_tensor("b2t", [C, H], F32, kind="ExternalInput")
    fcw_d = nc.dram_tensor("fcw", [C, H, 2], F16, kind="ExternalInput")
    fcb_d = nc.dram_tensor("fcb", [128, 2], F32, kind="ExternalInput")

    out_d = nc.dram_tensor("out", [IC, 2], F32, kind="ExternalOutput")

    with tile.TileContext(nc) as tc, ExitStack() as st:
        dram = st.enter_context(tc.tile_pool(name="dram", bufs=1,
                                             space="DRAM"))
        mn_dram = dram.tile([T, 128, IC], F16)
        edt_dram = dram.tile([H * IC], F16)
        hg_dram = dram.tile([IC, ROWW], F16)
        ag_out = dram.tile([KP, ROWW], F16,
                           addr_space=("Local" if fake_ag else "Shared"))

        const = st.enter_context(tc.tile_pool(name="const", bufs=1))
        sel5_sb = const.tile([5, T * 128], F32)
        own5ge_sb = const.tile([5, IC + H], F32)
        own3_sb = const.tile([3, IC], F32)
        agidx_sb = const.tile([128, T], I32)
        expdst_sb = const.tile([128, IC // 128], I32)
        w1p_sb = const.tile([3, HC], F32)
        w1d_sb = const.tile([3, H], F32)
        w2p_sb = const.tile([128, 2, HCE], F16)
        admw2_sb = const.tile([128, 2, H], F16)
        b1t_sb = const.tile([C, H], F32)
        b2t_sb = const.tile([C, H], F32)
        fcw_sb = const.tile([C, H, 2], F16)
        fcb_sb = const.tile([128, 2], F32)

        nc.sync.dma_start(out=own3_sb[:, :], in_=own3_d[:, :])
        nc.sync.dma_start(out=w1d_sb[:, :], in_=w1d_d[:, :])
        nc.sync.dma_start(out=sel5_sb[:, :], in_=sel5_d[:, :])
        nc.sync.dma_start(out=own5ge_sb[:, :], in_=own5ge_d[:, :])
        nc.sync.dma_start(out=w1p_sb[:, :], in_=w1p_d[:, :])
        nc.sync.dma_start(out=agidx_sb[:, :], in_=agidx_d[:, :])
        nc.sync.dma_start(out=expdst_sb[:, :], in_=expdst_d[:, :])
        nc.sync.dma_start(out=w2p_sb[:, :, :],
                          in_=w2p_d.rearrange("(s p) c -> p s c", p=128))
        nc.sync.dma_start(out=admw2_sb[:, :, :],
                          in_=admw2_d.rearrange("(s p) h -> p s h", p=128))
        nc.sync.dma_start(out=b1t_sb[:, :], in_=b1t_d[:, :])
        nc.sync.dma_start(out=b2t_sb[:, :], in_=b2t_d[:, :])
        nc.sync.dma_start(out=fcw_sb[:, :, :], in_=fcw_d[:, :, :])
        nc.sync.dma_start(out=fcb_sb[:, :], in_=fcb_d[:, :])

        big = st.enter_context(tc.tile_pool(name="big", bufs=1))
        # layer-1 source features, AG-row layout [h0|1|h1|1|h2|1|h3|1|es4]
        hsrc = big.tile([128, T, ROWW], F16)
        es4f = big.tile([128, T, H], F32)
        ed_b = big.tile([128, H, IC], F16)
        edt_sb = big.tile([H, IC], F16)
        edt_row = big.tile([1, H, IC], F16)
        x1T = big.tile([128, 2, IC], F16)
        x2T = big.tile([128, 2, IC], F16)
        hg_sb = big.tile([128, IC // 128, ROWW], F16)
        xr = big.tile([C, H, IC], F16)
        den_sb = big.tile([128, H * IC], F32)
        dinv_sb = big.tile([128, H * IC], F32)
        dinv_row = big.tile([1, H * IC], F32)
        dinv_b = big.tile([128, H, IC], F32)
        logit_sb = big.tile([128, IC // 128, 2], F32)

        h65 = hsrc[:, :, 0:H * (C + 1)].rearrange("p t (h x) -> p t h x", h=H)
        nc.vector.memset(h65[:, :, :, C:C + 1], 1.0)
        g65 = hg_sb[:, :, 0:H * (C + 1)].rearrange("p q (h x) -> p q h x",
                                                   h=H)
        nc.vector.memset(g65[:, :, :, C:C + 1], 1.0)

        for layer in (1, 2):
            # ---- prep: edt rows + partition-broadcast to ed_b ----
            with tc.tile_pool(name=f"prep{layer}", bufs=1,
                              space="PSUM") as prep_ps:
                edt_ps = prep_ps.tile([H, IC], F32, tag="edt")
                for lo, sz in ((0, 512), (512, 256)):
                    sl = slice(lo, lo + sz)
                    if layer == 1:
                        nc.tensor.matmul(edt_ps[:, sl], w1d_sb[:, :],
                                         own3_sb[:, sl],
                                         start=True, stop=True)
                    else:
                        for s2 in range(2):
                            nc.tensor.matmul(edt_ps[:, sl],
                                             admw2_sb[:, s2, :],
                                             x1T[:, s2, sl],
                                             start=(s2 == 0), stop=(s2 == 1))
                nc.scalar.copy(edt_sb[:, :], edt_ps[:, :])
            for h in range(H):
                nc.sync.dma_start(out=edt_row[0:1, h, :],
                                  in_=edt_sb[h:h + 1, :])
            for h in range(H):
                nc.gpsimd.partition_broadcast(ed_b[:, h, :],
                                              edt_row[0:1, h, :])

            # ---- slot loop ----
            with tc.tile_pool(name=f"agg_ps{layer}", bufs=1,
                              space="PSUM") as agg_pool:
                agg_ps = agg_pool.tile([128, H, IC], F32, tag="agg",
                                       name=f"agg_{layer}")
                with tc.tile_pool(name=f"jl{layer}", bufs=4) as jl, \
                     tc.tile_pool(name=f"mnp{layer}", bufs=3) as mnp, \
                     tc.tile_pool(name=f"h_ps{layer}", bufs=1,
                                  space="PSUM") as h_psp:
                    for s in range(T):
                        mn = mnp.tile([128, IC], F16, tag="mn",
                                      name=f"mn_{layer}_{s}")
                        if layer == 1:
                            h_ps = h_psp.tile([128, HC], F32, tag="h",
                                              name=f"h_ps_{s}")
                            nc.tensor.matmul(
                                h_ps[:, :],
                                sel5_sb[0:3, s * 128:(s + 1) * 128],
                                w1p_sb[:, :], start=True, stop=True)
                            nc.scalar.copy(
                                h65[:, s, :, 0:C],
                                h_ps[:, :].rearrange("p (h c) -> p h c",
                                                     h=H))
                            # g = R2-d2 (+es cols on 2nd chunk), 2 chunks
                            g_ps = h_psp.tile([128, GA + H], F32, tag="g",
                                              name=f"g_ps_{s}")
                            nc.tensor.matmul(
                                g_ps[:, 0:GA],
                                sel5_sb[:, s * 128:(s + 1) * 128],
                                own5ge_sb[:, 0:GA], start=True, stop=True)
                            nc.vector.tensor_scalar(
                                mn[:, 0:GA], g_ps[:, 0:GA], 0.0, MNEG,
                                OP.is_lt, OP.mult)
                            nc.tensor.matmul(
                                g_ps[:, :],
                                sel5_sb[:, s * 128:(s + 1) * 128],
                                own5ge_sb[:, GA:IC + H],
                                start=True, stop=True)
                            nc.vector.tensor_scalar(
                                mn[:, GA:IC], g_ps[:, 0:IC - GA], 0.0, MNEG,
                                OP.is_lt, OP.mult)
                            nc.vector.tensor_scalar_add(
                                es4f[:, s, :],
                                g_ps[:, IC - GA:IC - GA + H], 0.0)
                            nc.sync.dma_start(out=mn_dram[s, :, :],
                                              in_=mn[:, :])
                            src = hsrc[:, s, :]
                            es_ap = es4f[:, s, :]
                        else:
                            if s < IC // 128:
                                # own-node slots: sources are exactly this
                                # core's nodes s*128..s*128+127, whose h2
                                # rows already sit in hg_sb -- no gather,
                                # no dependency on the AllGather.
                                src = hg_sb[:, s, :]
                            else:
                                src = jl.tile([128, ROWW], F16, tag="hg",
                                              name=f"hg_{s}")
                                nc.gpsimd.indirect_dma_start(
                                    out=src[:, :], out_offset=None,
                                    in_=ag_out[:, :],
                                    in_offset=bass.IndirectOffsetOnAxis(
                                        ap=agidx_sb[:, s:s + 1], axis=0))
                            nc.sync.dma_start(out=mn[:, :],
                                              in_=mn_dram[s, :, :])
                            esg = jl.tile([128, H], F32, tag="esg",
                                          name=f"esg_{s}")
                            nc.vector.tensor_scalar_add(
                                esg[:, :],
                                src[:, H * (C + 1):ROWW], 0.0)
                            es_ap = esg[:, :]

                        # scores: L = leaky02(ed + es + mn); A = exp(L).
                        # u4 = ed + mn in ONE 2x TT via a stride-0 head
                        # broadcast of mn.  Then heads 0-1 get es+leaky via
                        # ACT Prelu (bias=es); heads 2-3 via 4x TS es-adds
                        # and a TS/TT leaky (STT only has a 1x uop).
                        L4 = jl.tile([128, H, IC], F16, tag="L4",
                                     name=f"L4_{layer}_{s}")
                        u4 = jl.tile([128, H, IC], F16, tag="u4",
                                     name=f"u4_{layer}_{s}")
                        ub, mb = bass.broadcast_tensor_aps(
                            ed_b[:, :, :],
                            mn[:, :].rearrange("p (o d) -> p o d", o=1))
                        nc.vector.tensor_tensor(u4[:, :, :], ub, mb, OP.add)
                        for h in range(2):
                            nc.scalar.activation(
                                L4[:, h, :], u4[:, h, :], AF.Prelu,
                                bias=es_ap[:, h:h + 1],
                                scale=1.0, alpha=0.2)
                        v2 = jl.tile([128, 2, IC], F16, tag="v2",
                                     name=f"v2_{layer}_{s}")
                        for h in range(2, H):
                            nc.vector.tensor_scalar_add(
                                v2[:, h - 2, :], u4[:, h, :],
                                es_ap[:, h:h + 1])
                        t2 = jl.tile([128, 2, IC], F16, tag="t2",
                                     name=f"t2_{layer}_{s}")
                        nc.vector.tensor_scalar_mul(t2[:, :, :], v2[:, :, :],
                                                    0.2)
                        nc.vector.tensor_tensor(L4[:, 2:4, :], v2[:, :, :],
                                                t2[:, :, :], OP.max)
                        A4 = jl.tile([128, H, IC], F16, tag="A4",
                                     name=f"A4_{layer}_{s}")
                        nc.scalar.activation(A4[:, :, :], L4[:, :, :], AF.Exp)
                        if dbg and layer == 1 and s == 0:
                            nc.sync.dma_start(out=dbg_d["dbg_mn0"][:, :],
                                              in_=mn[:, :])
                            nc.sync.dma_start(out=dbg_d["dbg_A0"][:, :, :],
                                              in_=A4[:, :, :])

                        # transposed aggregation: [h|ones] stationary.
                        # 256-col (1KB) chunks keep every matmul output
                        # inside one PSUM bank (head stride is 3KB).
                        # start=True clears has_written for the WHOLE bank,
                        # so only the first-issued region of each bank may
                        # set it (those with q ≡ h mod 2); the bank-mate
                        # region's first write then lands in overwrite mode.
                        for h in range(H):
                            for q in range(3):
                                lo = q * 256
                                nc.tensor.matmul(
                                    agg_ps[0:C + 1, h, lo:lo + 256],
                                    src[:, h * (C + 1):(h + 1) * (C + 1)],
                                    A4[:, h, lo:lo + 256],
                                    start=(s == 0 and (q % 2) == (h % 2)),
                                    stop=(s == T - 1))

                # ---- finalize: x^T = relu(num*dinv + b) ----
                bt_sb = b1t_sb if layer == 1 else b2t_sb
                xT = x1T if layer == 1 else x2T
                with tc.tile_pool(name=f"fin{layer}", bufs=1) as fin:
                    # den: PSUM -> SBUF row -> [128,:] for a lane-parallel
                    # exact reciprocal -> partition-0 row -> broadcast.
                    # Processed per head-pair so the two halves pipeline.
                    xc = fin.tile([C, H, IC], F16, tag="xc")
                    HF = 2 * IC  # elements per head-pair
                    for g2 in range(2):
                        hs = slice(2 * g2, 2 * g2 + 2)
                        fs = slice(g2 * HF, (g2 + 1) * HF)
                        # 1/den as exp(-ln(den)): two ACT table ops straight
                        # from PSUM beat the DVE iterative divide ~5x here
                        nc.scalar.activation(
                            den_sb[C:C + 1, fs],
                            agg_ps[C:C + 1, hs, :].rearrange(
                                "p h d -> p (h d)"),
                            AF.Ln)
                        nc.scalar.activation(
                            dinv_sb[C:C + 1, fs], den_sb[C:C + 1, fs],
                            AF.Exp, scale=-1.0)
                        nc.sync.dma_start(out=dinv_row[0:1, fs],
                                          in_=dinv_sb[C:C + 1, fs])
                        nc.gpsimd.partition_broadcast(
                            dinv_b[0:C, hs, :].rearrange(
                                "p h d -> p (h d)"),
                            dinv_row[0:1, fs])
                        nc.vector.tensor_tensor(
                            xc[:, hs, :], agg_ps[0:C, hs, :],
                            dinv_b[0:C, hs, :], OP.mult)
                        for h in range(2 * g2, 2 * g2 + 2):
                            nc.vector.tensor_scalar(
                                xr[:, h, :], xc[:, h, :], bt_sb[:, h:h + 1],
                                0.0, OP.add, OP.max)
                            po = (h % 2) * C
                            nc.sync.dma_start(
                                out=xT[po:po + C, h // 2, :],
                                in_=xr[0:C, h, :])

            if dbg and layer == 1:
                nc.sync.dma_start(out=dbg_d["dbg_x1T"][:, :, :],
                                  in_=x1T[:, :, :])
                nc.sync.dma_start(out=dbg_d["dbg_edb"][:, :, :],
                                  in_=ed_b[:, :, :])
                nc.sync.dma_start(out=dbg_d["dbg_hsrc"][:, :, :],
                                  in_=hsrc[:, :, :])
            if layer == 1:
                # ---- h2 rows (+es) for all own nodes; AllGather ----
                with tc.tile_pool(name="h2", bufs=1, space="PSUM") as h2p:
                    # half 0 (heads 0-1 of x1T) is ready after the first
                    # finalize half -- issue all 6 chunks' first matmuls
                    # immediately, accumulate half 1 when it lands
                    h2_tiles = [h2p.tile([128, HCE], F32, tag=f"h2_{oc}",
                                         name=f"h2_{oc}")
                                for oc in range(IC // 128)]
                    for s2 in range(2):
                        for oc in range(IC // 128):
                            nc.tensor.matmul(
                                h2_tiles[oc][:, :],
                                x1T[:, s2, oc * 128:(oc + 1) * 128],
                                w2p_sb[:, s2, :],
                                start=(s2 == 0), stop=(s2 == 1))
                    for oc in range(IC // 128):
                        nc.scalar.copy(
                            g65[:, oc, :, 0:C],
                            h2_tiles[oc][:, 0:HC].rearrange(
                                "p (h c) -> p h c", h=H))
                        nc.vector.tensor_scalar_add(
                            hg_sb[:, oc, H * (C + 1):ROWW],
                            h2_tiles[oc][:, HC:HCE], 0.0)
                nc.sync.dma_start(
                    out=hg_dram.rearrange("(q p) r -> p q r", p=128),
                    in_=hg_sb[:, :, :])
                if fake_ag:
                    for r in range(n_cores):
                        nc.sync.dma_start(
                            out=ag_out[r * IC:(r + 1) * IC, :],
                            in_=hg_dram[:, :])
                else:
                    nc.gpsimd.collective_compute(
                        "AllGather", OP.bypass,
                        replica_groups=[list(range(n_cores))],
                        ins=[hg_dram.opt()],
                        outs=[ag_out.opt()])
            else:
                # ---- fc head: per (head, chunk) straight from xr ----
                with tc.tile_pool(name="fc", bufs=1, space="PSUM") as fcp:
                    logit_ps = fcp.tile([128, IC // 128, 2], F32, tag="lg")
                    # all chunks share one PSUM bank: single start=True
                    for h in range(H):
                        for oc in range(IC // 128):
                            nc.tensor.matmul(
                                logit_ps[:, oc, :],
                                xr[0:C, h, oc * 128:(oc + 1) * 128],
                                fcw_sb[:, h, :],
                                start=(h == 0 and oc == 0), stop=(h == H - 1))
                    for o in range(2):
                        nc.vector.tensor_scalar_add(
                            logit_sb[:, :, o], logit_ps[:, :, o],
                            fcb_sb[:, o:o + 1])
                nc.sync.dma_start(
                    out=out_d.rearrange("(q p) o -> p q o", p=128),
                    in_=logit_sb[:, :, :])

    nc.compile()
    return nc


_BUILD_CACHE = {}


def _get_nc(nslot, nexp):
    key = (nslot, nexp)
    if key not in _BUILD_CACHE:
        _BUILD_CACHE[key] = build(nslot, nexp)
    return _BUILD_CACHE[key]


def _morton(p, bits=10):
    q = np.clip((p * (1 << bits)).astype(np.int64), 0, (1 << bits) - 1)
    code = np.zeros(len(p), np.int64)
    for b in range(bits):
        for dim in range(3):
            code |= ((q[:, dim] >> b) & 1) << (3 * b + dim)
    return code


def _plan(pts):
    """Sort nodes spatially; pick each core's relevant-source node list."""
    order = np.argsort(_morton(pts), kind="stable")
    p_sorted = np.full((KP, 3), PAD_COORD, np.float32)
    p_sorted[:K] = pts[order]

    sq = (p_sorted ** 2).sum(-1, dtype=np.float32)
    G = p_sorted @ p_sorted.T
    d2 = sq[None, :] + sq[:, None] - 2.0 * G
    near = d2 < (R2 + MASK_EPS)          # [src, dst], conservative superset

    srcs_list = []
    for c in range(N_CORES):
        srcs = np.flatnonzero(near[:, c * IC:(c + 1) * IC].any(axis=1))
        # own nodes first in identity order (they are always all present
        # via self-edges); remote sources after.  Slots 0..5 then read
        # their rows straight out of the resident hg_sb buffer.
        own = np.arange(c * IC, (c + 1) * IC, dtype=srcs.dtype)
        rem = srcs[(srcs < c * IC) | (srcs >= (c + 1) * IC)]
        srcs_list.append(np.concatenate([own, rem]))
    T = max(-(-len(s) // 128) for s in srcs_list)
    srcs_list = [np.concatenate(
        [s, np.full(T * 128 - len(s), PAD_NODE, s.dtype)])
        for s in srcs_list]
    # export sets: rows of owner o consumed by any other core
    exp_sets = [set() for _ in range(N_CORES)]
    for c in range(N_CORES):
        s = srcs_list[c]
        rem = s[(s != PAD_NODE) & ((s < c * IC) | (s >= (c + 1) * IC))]
        for r in rem:
            exp_sets[int(r) // IC].add(int(r))
    exp_rows = [np.array(sorted(e), np.int64) for e in exp_sets]
    E = max(8, max(len(e) for e in exp_rows))
    return order, p_sorted, srcs_list, T, exp_rows, E


def _blockdiag(a):  # [H, C] -> [HC, H] fp32
    m = np.zeros((HC, H), dtype=np.float32)
    for h in range(H):
        m[h * C:(h + 1) * C, h] = np.asarray(a, np.float32)[h]
    return m


def _prep_inputs(pos, pos_non_manifold, W1, a_src1, a_dst1, b1,
                 W2, a_src2, a_dst2, b2, fc_w, fc_b):
    f16 = np.float16
    pts = np.concatenate([np.asarray(pos, np.float32),
                          np.asarray(pos_non_manifold, np.float32)],
                         axis=2)[0].T  # [K, 3]
    order, p_sorted, srcs_list, T, exp_rows, E = _plan(pts)
    sq_sorted = (p_sorted ** 2).sum(-1, dtype=np.float32)
    # global node id -> AllGather row position (owner-block concat of exports)
    ag_pos = np.full(KP, 0, np.int64)
    for o in range(N_CORES):
        ag_pos[exp_rows[o]] = o * E + np.arange(len(exp_rows[o]))

    W1f = np.asarray(W1, np.float32)
    W2f = np.asarray(W2, np.float32)
    w1s = W1f @ _blockdiag(a_src1)            # [3, H]
    w2p = np.concatenate([W2f, W2f @ _blockdiag(a_src2)], axis=1)

    shared = {
        "w1p": np.ascontiguousarray(W1f),
        "w1d": np.ascontiguousarray(W1f @ _blockdiag(a_dst1)),
        "w2p": np.ascontiguousarray(w2p.astype(f16)),
        "admw2": np.ascontiguousarray(
            (W2f @ _blockdiag(a_dst2)).astype(f16)),
        "b1t": np.ascontiguousarray(
            np.asarray(b1, np.float32).reshape(H, C).T),
        "b2t": np.ascontiguousarray(
            np.asarray(b2, np.float32).reshape(H, C).T),
        "fcw": np.ascontiguousarray(np.asarray(fc_w, np.float32).reshape(
            H, C, 2).transpose(1, 0, 2).astype(f16)),
        "fcb": np.ascontiguousarray(np.broadcast_to(
            np.asarray(fc_b, np.float32).reshape(1, 2), (128, 2))),
    }
    in_maps = []
    for c in range(N_CORES):
        srcs = srcs_list[c]
        psel = p_sorted[srcs]                     # [T*128, 3]
        pown = p_sorted[c * IC:(c + 1) * IC]
        sel5 = np.concatenate(
            [psel.T, sq_sorted[srcs][None, :],
             np.ones((1, len(srcs)), np.float32)], axis=0)
        own5 = np.concatenate(
            [2.0 * pown.T, -np.ones((1, IC), np.float32),
             (R2 - sq_sorted[c * IC:(c + 1) * IC])[None, :]], axis=0)
        es_cols = np.concatenate(
            [w1s, np.zeros((2, H), np.float32)], axis=0)  # [5, H]
        m = dict(shared)
        m["sel5"] = np.ascontiguousarray(sel5)
        m["own5ge"] = np.ascontiguousarray(
            np.concatenate([own5, es_cols], axis=1))
        m["own3"] = np.ascontiguousarray(pown.T)
        m["agidx"] = np.ascontiguousarray(
            srcs.reshape(T, 128).T.astype(np.int32))
        m["expdst"] = np.zeros((128, IC // 128), np.int32)
        in_maps.append(m)
    return in_maps, order, T, E


def kernel(pos, pos_non_manifold, W1, a_src1, a_dst1, b1,
           W2, a_src2, a_dst2, b2, fc_w, fc_b, _trace=False):
    in_maps, order, T, E = _prep_inputs(
        pos, pos_non_manifold, W1, a_src1, a_dst1, b1,
        W2, a_src2, a_dst2, b2, fc_w, fc_b)
    nc = _get_nc(T, E)
    res = run_bass_kernel_spmd(nc, in_maps, core_ids=list(range(N_CORES)),
                               trace=_trace)
    kernel.last_results = res
    x2s = np.concatenate([res.results[c]["out"] for c in range(N_CORES)],
                         axis=0)  # [KP, 2] in sorted order
    x2 = np.empty((K, 2), np.float32)
    x2[order] = x2s[:K]
    logits = np.ascontiguousarray(x2[M:K]).reshape(1, 2, 3000)
    return logits.astype(np.float32)
